# revision 2
# baseline (speedup 1.0000x reference)
"""Trainium2 Bass kernel for a CMAE loss (masked reconstruction + contrastive).

Computes, for full inputs:
  reconstruct_loss = sum(mask * mean_P((pred - norm(target))^2)) / sum(mask)
      with norm(t) = (t - mean(t)) / sqrt(var_unbiased(t) + 1e-6)  per (b, l) row
  contrastive_loss = (sum_i logsumexp_j(S_ij/T) - trace(S)/T) / N
      with S = cos-sim matrix of row-normalized student/teacher [N, D]
  total = reconstruct_loss + contrastive_loss

Sharding: data-parallel over B across 8 NeuronCores (16 batches per core,
3136 rows of 768 pixels each); student/teacher (tiny) replicated, the
contrastive part computed identically on every core.  Each core emits a
[128, 4] stat tile (per-partition partials); the host sums partials and
forms the three scalars.

Per-core math (block-row layout, rows 24p+j on partition p):
  per row: bn_stats/bn_aggr give (m, vp) of t; Sp2 = sum(p^2) via ACT
  Square+accum; cross = sum((t - m) * p) via one scalar_tensor_tensor
  (per-partition scalar m) on the Pool engine.  Then with
  QE = P*vp + 767e-6, R = 1/QE, inv = sqrt(767*R):
  768*loss = Sp2 - 2*inv*cross + P*767*vp*R.
  Engine budget/body: DMA 54.4us (the roofline), DVE ~26us (bn_stats),
  ACT ~23us (Square), Pool ~29us (cross) -- compute hides under DMA.
"""

import numpy as np

B, L, P = 128, 196, 768
N, D = 128, 256
NCORES = 8
BSH = B // NCORES            # 16 batches per core
ROWS = BSH * L               # 3136 rows per core
NT = (ROWS + 127) // 128     # 25 stat columns (24 block-rows + remainder)
TEMP = 0.1
CP = float(P - 1)            # 767, unbiased-variance divisor
EPS_VAR = 1e-6

_CACHE = {}
ABLATE = set()    # {'dve','act','cross'}: skip recon-loop pieces (timing expts)
RPC = 2           # rows per partition per chunk DMA (bulk chunks)
TAIL1 = True      # split the last bulk chunk into two RPC=1 chunks
DMA_P = "sync"    # engine issuing pred loads: sync | scalar | gpsimd
CROSS_ENGINE = "gpsimd"  # engine for the (t-m)*p pass: gpsimd (Pool) | vector
DMA_OUT = "sync"  # engine issuing the final F store


def _build_program(repeat=1):
    import concourse.bacc as bacc
    import concourse.mybir as mybir
    import concourse.tile as tile
    from concourse.masks import make_identity

    class _Bacc(bacc.Bacc):
        """Bacc whose ACT-table chooser is restricted so every activation
        this kernel uses (Ln/Exp/Square/Copy/Identity) resolves to the one
        set that contains them all -- avoids ~6 ping-ponging table loads
        (~2.7us each) between natural_log / exp_and_others."""

        def insert_act_table_loads(self):
            from concourse.hw_specs import get_activation_tables
            import bass_rust as _br

            has_activation = any(
                isinstance(i, mybir.InstActivation)
                for b in self.main_func.blocks
                for i in b.instructions
            )
            if not has_activation:
                return
            mine = {
                mybir.ActivationFunctionType.Ln,
                mybir.ActivationFunctionType.Exp,
                mybir.ActivationFunctionType.Square,
                mybir.ActivationFunctionType.Copy,
                mybir.ActivationFunctionType.Identity,
            }
            keep = "natural_log_exp_and_others"
            tables = [
                (nm, (fs if nm == keep else (fs - mine)))
                for nm, fs in get_activation_tables(self.m.arch).items()
            ]
            _br.insert_act_table_loads(self, tables)

    f32 = mybir.dt.float32

    nc = _Bacc(
        "TRN2",
        target_bir_lowering=False,
        debug=False,
        enable_asserts=False,
    )
    tgt = nc.dram_tensor("target", [ROWS, P], f32, kind="ExternalInput").ap()
    prd = nc.dram_tensor("pred", [ROWS, P], f32, kind="ExternalInput").ap()
    msk = nc.dram_tensor("mask", [ROWS], f32, kind="ExternalInput").ap()
    stu = nc.dram_tensor("student", [N, D], f32, kind="ExternalInput").ap()
    tea = nc.dram_tensor("teacher", [N, D], f32, kind="ExternalInput").ap()
    out = nc.dram_tensor("out", [128, 4], f32, kind="ExternalOutput").ap()

    from contextlib import ExitStack

    with tile.TileContext(nc) as tc:
        with ExitStack() as ctx:
            consts = ctx.enter_context(tc.tile_pool(name="consts", bufs=1))
            accs = ctx.enter_context(tc.tile_pool(name="accs", bufs=1))
            io_t = ctx.enter_context(tc.tile_pool(name="io_t", bufs=4))
            io_p = ctx.enter_context(tc.tile_pool(name="io_p", bufs=4))
            scr_v = ctx.enter_context(tc.tile_pool(name="scr_v", bufs=2))
            scr_a = ctx.enter_context(tc.tile_pool(name="scr_a", bufs=2))
            scr_x = ctx.enter_context(tc.tile_pool(name="scr_x", bufs=2))
            small = ctx.enter_context(tc.tile_pool(name="small", bufs=2))
            epi = ctx.enter_context(tc.tile_pool(name="epi", bufs=1))
            psum = ctx.enter_context(tc.tile_pool(name="psum", bufs=2, space="PSUM"))
            ident = consts.tile([128, 128], f32)
            make_identity(nc, ident)
            zb = consts.tile([128, 1], f32)
            nc.gpsimd.memset(zb, 0.0)
            lnT = consts.tile([128, 1], f32)
            nc.gpsimd.memset(lnT, float(np.log(1.0 / TEMP)))

            for _rep in range(repeat):
                _run_body(
                    nc, tc, consts, accs, io_t, io_p, scr_v, scr_a, scr_x, small,
                    epi, psum, tgt, prd, msk, stu, tea, out, ident, zb, lnT,
                    mybir,
                )
    nc.compile()
    return nc


def _run_body(nc, tc, consts, accs, io_t, io_p, scr_v, scr_a, scr_x, small, epi,
              psum, tgt, prd, msk, stu, tea, out, ident, zb, lnT, mybir):
    import numpy as np

    f32 = mybir.dt.float32
    Alu = mybir.AluOpType
    Act = mybir.ActivationFunctionType
    X = mybir.AxisListType.X

    # F columns: 0=masked-loss partial, 1=mask partial, 2=lse, 3=diag
    F = accs.tile([128, 4], f32)
    nc.gpsimd.memset(F, 0.0)
    mv = accs.tile([128, NT, 2], f32)      # per-tile (mean, var) of t
    nc.gpsimd.memset(mv, 0.0)
    cross = accs.tile([128, NT], f32)      # sum((t - m) * p) per row
    nc.gpsimd.memset(cross, 0.0)
    s_p2 = accs.tile([128, NT], f32)
    nc.gpsimd.memset(s_p2, 0.0)
    mask_sb = accs.tile([128, NT], f32)
    nc.gpsimd.memset(mask_sb, 0.0)

    half = P // 2
    RPB = ROWS // 128                   # 24 rows per partition
    REM = ROWS - 128 * RPB              # 64 remainder rows
    tgt_blk = tgt[0 : 128 * RPB].rearrange("(p j) d -> p j d", j=RPB)
    prd_blk = prd[0 : 128 * RPB].rearrange("(p j) d -> p j d", j=RPB)
    p_dma = getattr(nc, DMA_P)
    x_eng = getattr(nc, "gpsimd" if CROSS_ENGINE == "gpsimd" else "vector")

    def compute_slice(t_ap, p_ap, col, h=128):
        """Stats for one [h, 768] slice: bn_stats/aggr -> mv[:, col, :],
        cross -> cross[:, col], Square+accum -> s_p2[:, col]."""
        if "dve" not in ABLATE:
            st = scr_v.tile([128, 2, 6], f32, tag="bn")
            nc.vector.bn_stats(st[:h, 0, :], t_ap[:, 0:half])
            nc.vector.bn_stats(st[:h, 1, :], t_ap[:, half:P])
            nc.vector.bn_aggr(mv[:h, col, :], st[:h])
        if "cross" not in ABLATE:
            sx = scr_x.tile([128, P], f32, tag="sx")
            x_eng.scalar_tensor_tensor(
                out=sx[:h], in0=t_ap, scalar=mv[:h, col, 0:1], in1=p_ap,
                op0=Alu.subtract, op1=Alu.mult,
                accum_out=cross[:h, col : col + 1],
            )
        if "act" not in ABLATE:
            sa = scr_a.tile([128, P], f32, tag="sa")
            nc.scalar.activation(
                sa[:h], p_ap, Act.Square, bias=zb[:h],
                accum_out=s_p2[:h, col : col + 1],
            )

    # ---- remainder rows first (their compute overlaps the bulk stream) ----
    if REM:
        h = REM
        t_r = io_t.tile([128, P], f32, tag="tr")
        nc.sync.dma_start(out=t_r[:h], in_=tgt[128 * RPB : ROWS, :])
        p_r = io_p.tile([128, P], f32, tag="pr")
        p_dma.dma_start(out=p_r[:h], in_=prd[128 * RPB : ROWS, :])
        compute_slice(t_r[:h], p_r[:h], RPB, h=h)

    # ---- bulk: block-row layout, rows 24p+j on partition p ----
    # Each chunk DMA moves `rpc` rows per partition as ONE contiguous
    # rpc*3072B descriptor per partition line (what the DMA engines need to
    # reach full HBM bandwidth).  The final chunks are single-row so the
    # post-DMA compute tail is as short as possible.
    chunks = []
    j0 = 0
    while j0 < RPB:
        rpc = RPC
        if TAIL1 and RPB - j0 <= 2 * RPC and RPB - j0 > 1:
            rpc = 1
        rpc = min(rpc, RPB - j0)
        chunks.append((j0, rpc))
        j0 += rpc

    for c, (j0, rpc) in enumerate(chunks):
        if c == 2:
            # mask in block-row layout: mask_sb[p, j] = mask[RPB*p + j]
            nc.sync.dma_start(
                out=mask_sb[:, 0:RPB],
                in_=msk[0 : RPB * 128].rearrange("(p j) -> p j", j=RPB),
            )
            if REM:
                nc.sync.dma_start(
                    out=mask_sb[0:REM, RPB : RPB + 1],
                    in_=msk[RPB * 128 : ROWS].rearrange("(p j) -> p j", j=1),
                )

            # ---- contrastive part (tiny, replicated on every core) ----
            stu_sb = consts.tile([N, D], f32)
            nc.sync.dma_start(out=stu_sb, in_=stu)
            tea_sb = consts.tile([N, D], f32)
            nc.sync.dma_start(out=tea_sb, in_=tea)

            qs = small.tile([128, 1], f32)
            qt = small.tile([128, 1], f32)
            c_scr = small.tile([N, D], f32)
            nc.vector.scalar_tensor_tensor(
                out=c_scr, in0=stu_sb, scalar=1.0, in1=stu_sb,
                op0=Alu.mult, op1=Alu.mult, accum_out=qs,
            )
            c_scr2 = small.tile([N, D], f32)
            nc.vector.scalar_tensor_tensor(
                out=c_scr2, in0=tea_sb, scalar=1.0, in1=tea_sb,
                op0=Alu.mult, op1=Alu.mult, accum_out=qt,
            )
            # 1/||row|| = exp(-0.5*ln(q)); student side also folds in 1/T=10
            lnqs = small.tile([128, 1], f32)
            nc.scalar.activation(lnqs, qs, Act.Ln, bias=zb)
            lnqt = small.tile([128, 1], f32)
            nc.scalar.activation(lnqt, qt, Act.Ln, bias=zb)
            a10 = small.tile([128, 1], f32)
            nc.scalar.activation(a10, lnqs, Act.Exp, scale=-0.5, bias=lnT)
            b1 = small.tile([128, 1], f32)
            nc.scalar.activation(b1, lnqt, Act.Exp, scale=-0.5, bias=zb)

            PN = consts.tile([N, D], f32)
            nc.vector.tensor_scalar(
                out=PN, in0=stu_sb, scalar1=a10, scalar2=None, op0=Alu.mult
            )
            TN = consts.tile([N, D], f32)
            nc.vector.tensor_scalar(
                out=TN, in0=tea_sb, scalar1=b1, scalar2=None, op0=Alu.mult
            )
            # diag of S: row-dots of the scaled matrices -> F[:, 3]
            c_scr3 = small.tile([N, D], f32)
            nc.vector.scalar_tensor_tensor(
                out=c_scr3, in0=PN, scalar=1.0, in1=TN,
                op0=Alu.mult, op1=Alu.mult, accum_out=F[:, 3:4],
            )

            # S = PN @ TN.T via PE: transpose both, then 2 accumulating matmuls
            nchunks = D // 128
            pnt = []
            tnt = []
            for cc in range(nchunks):
                for src, dstlist, nm in ((PN, pnt, "pn"), (TN, tnt, "tn")):
                    ps = psum.tile([128, 128], f32, tag="tr_ps")
                    nc.tensor.transpose(ps, src[:, cc * 128 : (cc + 1) * 128], ident)
                    sb = consts.tile([128, 128], f32, tag=f"{nm}t{cc}")
                    nc.scalar.copy(sb, ps)
                    dstlist.append(sb)
            S_ps = psum.tile([128, 128], f32, tag="S")
            for cc in range(nchunks):
                nc.tensor.matmul(
                    S_ps, lhsT=pnt[cc], rhs=tnt[cc],
                    start=(cc == 0), stop=(cc == nchunks - 1),
                )
            # row-wise logsumexp -> F[:, 2]
            rm_neg = small.tile([128, 1], f32)
            nc.vector.tensor_reduce(rm_neg, S_ps, axis=X, op=Alu.max, negate=True)
            E = small.tile([128, 128], f32)
            sume = small.tile([128, 1], f32)
            nc.scalar.activation(E, S_ps, Act.Exp, bias=rm_neg, accum_out=sume)
            lnsum = small.tile([128, 1], f32)
            nc.scalar.activation(lnsum, sume, Act.Ln, bias=zb)
            nc.vector.tensor_sub(F[:, 2:3], lnsum, rm_neg)

        t_t = io_t.tile([128, RPC, P], f32, tag="t")
        nc.sync.dma_start(out=t_t[:, 0:rpc, :], in_=tgt_blk[:, j0 : j0 + rpc, :])
        p_t = io_p.tile([128, RPC, P], f32, tag="p")
        p_dma.dma_start(out=p_t[:, 0:rpc, :], in_=prd_blk[:, j0 : j0 + rpc, :])
        for jj in range(rpc):
            compute_slice(t_t[:, jj, :], p_t[:, jj, :], j0 + jj)

    # ---- per-row loss epilogue on the [128, NT] stat buffers ----
    m_ap = mv[:, :, 0]
    vp_ap = mv[:, :, 1]
    QE = epi.tile([128, NT], f32)   # q + 767*eps, q = P*var_pop
    nc.vector.tensor_scalar(
        out=QE, in0=vp_ap, scalar1=float(P), scalar2=CP * EPS_VAR,
        op0=Alu.mult, op1=Alu.add,
    )
    R = epi.tile([128, NT], f32)
    nc.vector.reciprocal(R, QE)
    # inv = sqrt(767*R) = exp(0.5*ln(767*R))
    LNR = epi.tile([128, NT], f32)
    nc.scalar.activation(LNR, R, Act.Ln, scale=CP, bias=zb)
    INV = epi.tile([128, NT], f32)
    nc.scalar.activation(INV, LNR, Act.Exp, scale=0.5, bias=zb)
    T1 = epi.tile([128, NT], f32)   # -2 * inv * cross
    nc.vector.scalar_tensor_tensor(
        out=T1, in0=cross, scalar=-2.0, in1=INV, op0=Alu.mult, op1=Alu.mult
    )
    T2 = epi.tile([128, NT], f32)
    nc.vector.tensor_add(T2, T1, s_p2)
    T3 = epi.tile([128, NT], f32)
    nc.vector.tensor_mul(T3, vp_ap, R)
    T4 = epi.tile([128, NT], f32)   # = 768 * per-row loss
    nc.vector.scalar_tensor_tensor(
        out=T4, in0=T3, scalar=float(P) * CP, in1=T2,
        op0=Alu.mult, op1=Alu.add,
    )
    LM = epi.tile([128, NT], f32)
    nc.vector.scalar_tensor_tensor(
        out=LM, in0=T4, scalar=1.0 / P, in1=mask_sb,
        op0=Alu.mult, op1=Alu.mult, accum_out=F[:, 0:1],
    )
    nc.vector.tensor_reduce(F[:, 1:2], mask_sb, axis=X, op=Alu.add)

    # ---- store the per-partition stat tile; the host does the final sums ----
    getattr(nc, DMA_OUT).dma_start(out=out, in_=F)


def _get_program(repeat=1):
    key = ("nc", repeat, tuple(sorted(ABLATE)), RPC, TAIL1, DMA_P, CROSS_ENGINE,
           DMA_OUT)
    if key not in _CACHE:
        _CACHE[key] = _build_program(repeat)
    return _CACHE[key]


def _shard_inputs(student_prob, teacher_prob, reconstruct_target, reconstruct_pred, mask):
    student = np.ascontiguousarray(student_prob, dtype=np.float32)
    teacher = np.ascontiguousarray(teacher_prob, dtype=np.float32)
    tgt = np.ascontiguousarray(reconstruct_target, dtype=np.float32)
    prd = np.ascontiguousarray(reconstruct_pred, dtype=np.float32)
    msk = np.ascontiguousarray(mask, dtype=np.float32)

    in_maps = []
    for c in range(NCORES):
        sl = slice(c * BSH, (c + 1) * BSH)
        in_maps.append(
            {
                "target": tgt[sl].reshape(ROWS, P),
                "pred": prd[sl].reshape(ROWS, P),
                "mask": msk[sl].reshape(ROWS),
                "student": student,
                "teacher": teacher,
            }
        )
    return in_maps


def _combine(results):
    outs = np.stack([r["out"] for r in results])  # [NCORES, 128, 4]
    num = float(outs[:, :, 0].sum())
    den = float(outs[:, :, 1].sum())
    recon = num / den
    contr = (float(outs[0, :, 2].sum()) - float(outs[0, :, 3].sum())) / N
    total = recon + contr
    return (np.float32(recon), np.float32(contr), np.float32(total))


def run(in_maps, repeat=1, **kwargs):
    from concourse.bass_utils import run_bass_kernel_spmd

    nc = _get_program(repeat)
    return run_bass_kernel_spmd(nc, in_maps, core_ids=list(range(NCORES)), **kwargs)


def kernel(student_prob, teacher_prob, reconstruct_target, reconstruct_pred, mask):
    in_maps = _shard_inputs(
        student_prob, teacher_prob, reconstruct_target, reconstruct_pred, mask
    )
    res = run(in_maps)
    return _combine(res.results)
